# revision 18
# baseline (speedup 1.0000x reference)
"""Trainium2 Bass kernel for nn_DeltaAI_84061099918079 (gnn_message_passing).

Math reformulation of the reference:
  For each batch row b with i = ilist[b], the 9 qnet evaluations (1 self +
  8 children) all use Vin = V[b] * M[v] where M[v, c] = (c < 128 or
  c in K_pa[v]) is one of only 1024 distinct masks, and v = i (slot 0) or
  v = K_ch[i, s-1] (slots 1..8).  bern_logprob(q, t) == t*q - softplus(q).
  elu(x) == relu(x) + min(exp(x), 1) - 1.

Optimizations on top of the straightforward mapping:
  - Weight columns are centered host-side (W~ = W - colmean): the LN mean of
    x = W^T v + b is then exactly 0, so no mean stats and no mean subtract.
  - LN gain g is folded into W (stats selector carries 1/(H g^2)); LN beta and
    layer biases are zero in this problem so those adds are skipped (checked).
  - Activations are stored shifted (h' = h+1) so ELU needs no "-1" pass; the
    shift is compensated in the next layer's bias and the head bias table.
  - Matmuls run slot-inner so consecutive matmuls share Ldweights.
  - Elementwise work is spread: DVE uses 2x-mode tensor_tensor ops only,
    PSUM->SBUF copies and the L1 ELU combine run on GPSIMD/Pool.
"""

import os
import sys
import numpy as np

sys.path.insert(0, "/opt/trn_rl_repo")

import ml_dtypes

bf16 = ml_dtypes.bfloat16

B, VDIM, XDIM, HDIM = 4096, 1024, 128, 512
MAXPA, MAXCH = 8, 8
LN_EPS = 1e-5
NCORES = 8
BSH = B // NCORES          # 512 batch rows per core
NS = 1 + MAXCH             # 9 slots
N = BSH                    # tile columns
KC_V = VDIM // 128         # 8
KC_H = HDIM // 128         # 4

_PROGRAM = {}              # cached per structure-flags


def _build_program(bias_zero, be_zero):
    import concourse.bass as bass
    import concourse.mybir as mybir
    import concourse.tile as tile
    from concourse import bacc
    from contextlib import ExitStack

    FP32 = mybir.dt.float32
    BF16 = mybir.dt.bfloat16
    I16 = mybir.dt.int16
    AF = mybir.ActivationFunctionType
    ALU = mybir.AluOpType
    ts = bass.ts

    nc = bacc.Bacc("TRN2")

    # ---- DRAM tensors ----
    vt_d = nc.dram_tensor("vt", [128, KC_V, N], BF16, kind="ExternalInput")
    mrows_d = nc.dram_tensor("mrows", [VDIM, VDIM], BF16, kind="ExternalInput")
    hwrows_d = nc.dram_tensor("hwrows", [VDIM, HDIM], BF16, kind="ExternalInput")
    w1_d = nc.dram_tensor("w1", [128, KC_V, HDIM], BF16, kind="ExternalInput")
    w2_d = nc.dram_tensor("w2", [128, KC_H, HDIM], BF16, kind="ExternalInput")
    w3_d = nc.dram_tensor("w3", [128, KC_H, HDIM], BF16, kind="ExternalInput")
    if not bias_zero:
        bprm_d = nc.dram_tensor("bprm", [128, 3, KC_H], FP32, kind="ExternalInput")
    if not be_zero:
        beprm_d = nc.dram_tensor("beprm", [128, 3, KC_H], FP32, kind="ExternalInput")
    # stats selector: tile (li,k,j) has col j = 1/(H g^2) weights, rest zero
    selq_d = nc.dram_tensor("selq", [128, 3 * KC_H * 3, 16], BF16, kind="ExternalInput")
    idx_d = nc.dram_tensor("idx", [128, NS, N // 16], I16, kind="ExternalInput")
    tmat_d = nc.dram_tensor("tmat", [NS, N], FP32, kind="ExternalInput")
    mch_d = nc.dram_tensor("mch", [NS, N], FP32, kind="ExternalInput")
    hbg_d = nc.dram_tensor("hbg", [NS, N], FP32, kind="ExternalInput")
    sel_d = nc.dram_tensor("sel", [128, 2 * NS, 64], BF16, kind="ExternalInput")
    fin_d = nc.dram_tensor("fin", [16, 2], FP32, kind="ExternalInput")
    out_d = nc.dram_tensor("out", [2, N], FP32, kind="ExternalOutput")
    llout_d = nc.dram_tensor("llout", [NS, N], FP32, kind="ExternalOutput")

    with tile.TileContext(nc) as tc, ExitStack() as ctx:
        const = ctx.enter_context(tc.tile_pool(name="const", bufs=1))
        hA = ctx.enter_context(tc.tile_pool(name="hA", bufs=1))
        hB = ctx.enter_context(tc.tile_pool(name="hB", bufs=1))
        mgp = ctx.enter_context(tc.tile_pool(name="mgp", bufs=2))
        sqp = ctx.enter_context(tc.tile_pool(name="sqp", bufs=3))
        tmp = ctx.enter_context(tc.tile_pool(name="tmp", bufs=7))
        hwp = ctx.enter_context(tc.tile_pool(name="hwp", bufs=2))
        mbp = ctx.enter_context(tc.tile_pool(name="mbp", bufs=4))
        smp = ctx.enter_context(tc.tile_pool(name="smp", bufs=1))
        xps = ctx.enter_context(
            tc.tile_pool(name="xps", bufs=6, space=bass.MemorySpace.PSUM))
        stp = ctx.enter_context(
            tc.tile_pool(name="stp", bufs=1, space=bass.MemorySpace.PSUM))
        qps = ctx.enter_context(
            tc.tile_pool(name="qps", bufs=1, space=bass.MemorySpace.PSUM))

        # ---- load constants ----
        _eng = [nc.sync, nc.gpsimd, nc.scalar]
        _engi = [0]

        def load(shape, dt, src, tag):
            t = const.tile(shape, dt, tag=tag, name=tag)
            _eng[_engi[0] % len(_eng)].dma_start(t[:], src[:])
            _engi[0] += 1
            return t

        idxa = load([128, NS, N // 16], I16, idx_d, "idxa")
        vt = load([128, KC_V, N], BF16, vt_d, "vt")
        w1 = load([128, KC_V, HDIM], BF16, w1_d, "w1")
        w2 = load([128, KC_H, HDIM], BF16, w2_d, "w2")
        w3 = load([128, KC_H, HDIM], BF16, w3_d, "w3")
        selq = load([128, 3 * KC_H * 3, 16], BF16, selq_d, "selq")
        tmat = load([NS, N], FP32, tmat_d, "tmat")
        mch = load([NS, N], FP32, mch_d, "mch")
        hbg = load([NS, N], FP32, hbg_d, "hbg")
        sel = load([128, 2 * NS, 64], BF16, sel_d, "sel")
        fin = load([16, 2], FP32, fin_d, "fin")
        if not bias_zero:
            bprm = load([128, 3, KC_H], FP32, bprm_d, "bprm")
        if not be_zero:
            beprm = load([128, 3, KC_H], FP32, beprm_d, "beprm")
        idxt = [idxa[:, s, :] for s in range(NS)]
        epst = const.tile([NS, 1], FP32, tag="epst", name="epst")
        nc.vector.memset(epst[:], LN_EPS)
        onet = const.tile([NS, 1], FP32, tag="onet", name="onet")
        nc.vector.memset(onet[:], 1.0)
        onep = const.tile([128, 1], FP32, tag="onep", name="onep")
        nc.vector.memset(onep[:], 1.0)
        zt = const.tile([128, KC_H, N], BF16, tag="zt", name="zt")
        nc.vector.memset(zt[:], 0.0)

        ws = [w1, w2, w3]
        kcs = [KC_V, KC_H, KC_H]

        hAt = [hA.tile([128, KC_H, N], BF16, tag=f"hA{s}", name=f"hA{s}") for s in range(NS)]
        hBt = [hB.tile([128, KC_H, N], BF16, tag=f"hB{s}", name=f"hB{s}") for s in range(NS)]

        # ---- Phase 0: per-slot masked inputs vin = V^T * M[v]^T ----
        vin_t = []
        for s in range(NS):
            mg = mgp.tile([128, KC_V, N], BF16, tag="mg")
            nc.gpsimd.dma_gather(
                mg[:], mrows_d[:], idxt[s][:], N, N, VDIM, transpose=True)
            nc.vector.tensor_mul(mg[:], vt[:], mg[:])
            vin_t.append(mg)

        # ---- layers ----
        def run_layer(li, inputs, houts, hres):
            """x~ = W~g^T @ inputs (zero-mean by construction); per slot:
            h' = relu(y) + min(exp(y),1) [+ hres] with y = x~ * rstd."""
            w, kc = ws[li], kcs[li]
            for gi, grp in enumerate(([0, 1, 2], [3, 4, 5], [6, 7, 8])):
              # main matmuls
              for j, s in enumerate(grp):
                for m in range(KC_H):
                    xp = xps.tile([128, N], mybir.dt.float32, tag="xp",
                                  name=f"xp{li}{gi}{m}{j}")
                    for k in range(kc):
                        nc.tensor.matmul(
                            xp[:], w[:, k, ts(m, 128)], inputs[s][:, k, :],
                            start=(k == 0), stop=(k == kc - 1))
                    if bias_zero:
                        nc.scalar.activation(
                            houts[s][:, m, :], xp[:], AF.Identity)
                    else:
                        nc.scalar.activation(
                            houts[s][:, m, :], xp[:], AF.Identity,
                            bias=bprm[:, li, m:m + 1])

              # variance stats: E[x~^2] via selector matmuls (row j of stat)
              stat = stp.tile([16, N], mybir.dt.float32, tag="stat",
                              name=f"stat{li}{gi}")
              sqs = []
              for j, s in enumerate(grp):
                sq = sqp.tile([128, KC_H, N], BF16, tag="sq",
                              name=f"sq{li}{s}")
                nc.vector.tensor_mul(sq[:], houts[s][:], houts[s][:])
                sqs.append(sq)
              for j, s in enumerate(grp):
                for k in range(KC_H):
                    nc.tensor.matmul(
                        stat[:], selq[:, (li * KC_H + k) * 3 + j, :],
                        sqs[j][:, k, :],
                        start=(j == 0 and k == 0),
                        stop=(j == 2 and k == KC_H - 1),
                        skip_group_check=True)

              # rstd = exp(-0.5*ln(var+eps)) on [3, N] rows
              lnv = smp.tile([NS, N], mybir.dt.float32, tag="lnv",
                             name=f"lnv{li}{gi}")[0:3, :]
              nc.scalar.activation(lnv[:], stat[0:3, :], AF.Ln, bias=epst[0:3])
              mrp = smp.tile([NS, 1, N], BF16, tag="mrp",
                             name=f"mrp{li}{gi}")
              nc.scalar.activation(mrp[0:3, 0, :], lnv[:], AF.Exp, scale=-0.5)

              for j, s in enumerate(grp):
                xs = houts[s]
                mrps = mbp.tile([1, 1, N], BF16, tag="mrps",
                                name=f"mrps{li}{s}")
                nc.sync.dma_start(mrps[:], mrp[j:j + 1, :, :])
                mb = mbp.tile([128, 1, N], BF16, tag="mb")
                nc.gpsimd.partition_broadcast(mb[:], mrps[:])
                r_b = mb[:, 0:1, :].broadcast_to([128, KC_H, N])
                yy = tmp.tile([128, KC_H, N], BF16, tag="tmp")
                nc.vector.tensor_mul(yy[:], xs[:], r_b)
                if not be_zero:
                    for m in range(KC_H):
                        nc.gpsimd.tensor_scalar(
                            yy[:, m, :], yy[:, m, :],
                            beprm[:, li, m:m + 1], None, op0=ALU.add)
                ee = tmp.tile([128, KC_H, N], BF16, tag="tmp")
                nc.scalar.activation(ee[:], yy[:], AF.Exp)
                rl = tmp.tile([128, KC_H, N], BF16, tag="tmp")
                nc.vector.tensor_max(rl[:], yy[:], zt[:])
                if hres is None:
                    # h1' = min(ee,1) + relu(y)
                    nc.vector.scalar_tensor_tensor(
                        xs[:], ee[:], 1.0, rl[:], op0=ALU.min, op1=ALU.add)
                else:
                    # h' = hres' + relu(y) - relu(1-ee)
                    tt = tmp.tile([128, KC_H, N], BF16, tag="tmp")
                    nc.scalar.activation(tt[:], ee[:], AF.Relu,
                                         bias=onep[:], scale=-1.0)
                    vv = tmp.tile([128, KC_H, N], BF16, tag="tmp")
                    nc.vector.tensor_sub(vv[:], rl[:], tt[:])
                    nc.vector.tensor_add(xs[:], vv[:], hres[s][:])

        run_layer(0, vin_t, hAt, None)          # h1' in hAt
        run_layer(1, hAt, hBt, hAt)             # h2' in hBt
        run_layer(2, hBt, hAt, hBt)             # h3' in hAt

        # ---- head: q[s, n] = sum_h h3'[h,n]*headW[v][h]  (shift fixed in hbg)
        qp = qps.tile([32, N], mybir.dt.float32, tag="q")
        for s in range(NS):
            hw = hwp.tile([128, KC_H, N], BF16, tag="hw")
            nc.gpsimd.dma_gather(
                hw[:], hwrows_d[:], idxt[s][:], N, N, HDIM, transpose=True)
            nc.vector.tensor_mul(hw[:], hAt[s][:], hw[:])
            for k in range(KC_H):
                nc.tensor.matmul(
                    qp[:], sel[:, 6 + s, 0:32], hw[:, k, :],
                    start=(s == 0 and k == 0),
                    stop=(s == NS - 1 and k == KC_H - 1),
                    skip_group_check=True)

        # ---- bern ll + child sum ----
        q2 = smp.tile([NS, N], mybir.dt.float32, tag="q2")
        nc.vector.scalar_tensor_tensor(
            q2[:], qp[0:NS, :], 1.0, hbg[:], op0=ALU.mult, op1=ALU.add)
        # softplus(q) = relu(q) + ln(1 + exp(-|q|))
        aq = smp.tile([NS, N], mybir.dt.float32, tag="aq")
        nc.scalar.activation(aq[:], q2[:], AF.Abs)
        eq = smp.tile([NS, N], mybir.dt.float32, tag="eq")
        nc.scalar.activation(eq[:], aq[:], AF.Exp, scale=-1.0)
        lg = smp.tile([NS, N], mybir.dt.float32, tag="lg")
        nc.scalar.activation(lg[:], eq[:], AF.Ln, bias=onet[:])
        rq = smp.tile([NS, N], mybir.dt.float32, tag="rq")
        nc.vector.tensor_scalar_max(rq[:], q2[:], 0.0)
        sp = smp.tile([NS, N], mybir.dt.float32, tag="sp")
        nc.vector.tensor_add(sp[:], rq[:], lg[:])
        tq = smp.tile([NS, N], mybir.dt.float32, tag="tq")
        nc.vector.tensor_mul(tq[:], tmat[:], q2[:])
        llv = smp.tile([NS, N], mybir.dt.float32, tag="llv")
        nc.vector.scalar_tensor_tensor(
            llv[:], sp[:], -1.0, tq[:], op0=ALU.mult, op1=ALU.add)
        llm = const.tile([16, N], mybir.dt.float32, tag="llm")
        nc.vector.memset(llm[:], 0.0)
        nc.vector.tensor_mul(llm[0:NS, :], llv[:], mch[:])
        fo = qps.tile([32, N], mybir.dt.float32, tag="q", name="fo")[0:2, :]
        nc.tensor.matmul(fo[:], fin[:], llm[:], start=True, stop=True)
        ob = smp.tile([2, N], mybir.dt.float32, tag="ob")
        nc.vector.tensor_copy(ob[:], fo[:])
        nc.sync.dma_start(out_d[:], ob[:])
        nc.sync.dma_start(llout_d[:], llv[:])

    nc.compile()
    return nc


def _get_program(bias_zero=True, be_zero=True):
    key = (bias_zero, be_zero)
    if key not in _PROGRAM:
        _PROGRAM[key] = _build_program(bias_zero, be_zero)
    return _PROGRAM[key]


def _host_prep(V, K_pa, K_ch, ilist, W1, W2, W3, b1, g1, be1, b2, g2, be2,
               b3, g3, be3, headW, headb):
    """Index-derived tables + sharded/replicated device buffers."""
    V = np.asarray(V, np.float32)
    K_pa = np.asarray(K_pa).astype(np.int64)
    K_ch = np.asarray(K_ch).astype(np.int64)
    ilist = np.asarray(ilist).astype(np.int64)

    # mask matrix M[v, c] (bf16 exact 0/1)
    M = np.zeros((VDIM, VDIM), np.float32)
    M[:, :XDIM] = 1.0
    vr = np.repeat(np.arange(VDIM), MAXPA)
    pa = K_pa.ravel()
    ok = pa >= 0
    M[vr[ok], pa[ok]] = 1.0

    # node index per (slot, batch-row)
    vmat = np.zeros((NS, B), np.int64)
    vmat[0] = ilist
    ch = K_ch[ilist]                     # [B, 8]
    ch_ok = ch >= 0
    vmat[1:] = np.where(ch_ok, ch, 0).T  # [8, B]

    tmat = V[np.arange(B)[None, :], vmat].astype(np.float32)      # [NS, B]
    mch = np.ones((NS, B), np.float32)
    mch[1:] = ch_ok.T.astype(np.float32)
    # head bias with the h'=h+1 shift correction folded in
    hw_rs = np.asarray(headW, np.float32).sum(1)                  # [VDIM]
    hbg = (np.asarray(headb, np.float32) - hw_rs)[vmat]           # [NS, B]

    # center weight columns (LN mean -> 0), fold gain g into W, and fold the
    # h'=h+1 input shift of layers 2/3 into their (centered) biases
    gs = [np.asarray(g, np.float32) for g in (g1, g2, g3)]
    bs = [np.asarray(b, np.float32) for b in (b1, b2, b3)]
    bes = [np.asarray(be, np.float32) for be in (be1, be2, be3)]
    Wg, bias_l = [], []
    for li, W in enumerate((W1, W2, W3)):
        W = np.asarray(W, np.float32)
        Wc = (W - W.mean(1, keepdims=True)) * gs[li][None, :]
        bc = (bs[li] - bs[li].mean()) * gs[li]
        if li > 0:
            bc = bc - Wc.sum(0)          # input is h_prev + 1
        Wg.append(Wc)
        bias_l.append(bc)

    bias_zero = all(np.abs(b).max() < 1e-30 for b in bias_l)
    be_zero = all(np.abs(be).max() < 1e-30 for be in bes)

    def chunk_feat(w, kc):
        return np.ascontiguousarray(
            np.asarray(w, np.float32).reshape(kc, 128, -1).transpose(1, 0, 2)
        ).astype(bf16)

    w1c = chunk_feat(Wg[0], KC_V)
    w2c = chunk_feat(Wg[1], KC_H)
    w3c = chunk_feat(Wg[2], KC_H)

    def chunk_param(*ps):
        return np.ascontiguousarray(np.stack(
            [np.asarray(p, np.float32).reshape(KC_H, 128).T for p in ps],
            axis=1))

    bprm = chunk_param(*bias_l)
    beprm = chunk_param(*bes)

    # stats selector: tile (li,k,j) has only col j nonzero = 1/(H g^2)
    selq = np.zeros((128, 3 * KC_H * 3, 16), np.float32)
    for li in range(3):
        wv = 1.0 / (HDIM * gs[li] ** 2)              # [HDIM]
        for k in range(KC_H):
            for j in range(3):
                selq[:, (li * KC_H + k) * 3 + j, j] = wv[k * 128:(k + 1) * 128]

    sel = np.zeros((128, 2 * NS, 64), np.float32)
    for s in range(NS):
        sel[:, 6 + s, s] = 1.0
    fin = np.zeros((16, 2), np.float32)
    fin[0, 0] = 1.0
    fin[1:NS, 1] = 1.0

    Mb = M.astype(bf16)
    HWb = np.asarray(headW, np.float32).astype(bf16)

    in_maps = []
    for c in range(NCORES):
        rows = slice(c * BSH, (c + 1) * BSH)
        vt = np.ascontiguousarray(
            V[rows].T.reshape(KC_V, 128, BSH).transpose(1, 0, 2)).astype(bf16)
        vm = vmat[:, rows]                                        # [NS, 512]
        idx = np.zeros((128, NS, N // 16), np.int16)
        for s in range(NS):
            wrapped = vm[s].reshape(N // 16, 16).T.astype(np.int16)
            idx[:, s, :] = np.tile(wrapped, (8, 1))
        im = dict(
            vt=vt, mrows=Mb, hwrows=HWb, w1=w1c, w2=w2c, w3=w3c,
            selq=selq.astype(bf16), idx=idx,
            tmat=np.ascontiguousarray(tmat[:, rows]),
            mch=np.ascontiguousarray(mch[:, rows]),
            hbg=np.ascontiguousarray(hbg[:, rows]),
            sel=sel.astype(bf16), fin=fin,
        )
        if not bias_zero:
            im["bprm"] = bprm
        if not be_zero:
            im["beprm"] = beprm
        in_maps.append(im)

    aux = dict(M=M, vmat=vmat, tmat=tmat, mch=mch,
               bias_zero=bias_zero, be_zero=be_zero)
    return in_maps, aux


def kernel(V, K_pa, K_ch, ilist, W1, b1, g1, be1, W2, b2, g2, be2,
           W3, b3, g3, be3, headW, headb, marginals):
    from concourse.bass_utils import run_bass_kernel_spmd

    in_maps, aux = _host_prep(V, K_pa, K_ch, ilist, W1, W2, W3, b1, g1, be1,
                              b2, g2, be2, b3, g3, be3, headW, headb)
    nc = _get_program(aux["bias_zero"], aux["be_zero"])
    res = run_bass_kernel_spmd(nc, in_maps, core_ids=list(range(NCORES)))
    out = np.concatenate([r["out"] for r in res.results], axis=1)  # [2, B]
    llv = np.concatenate([r["llout"] for r in res.results], axis=1)  # [NS, B]

    # Exact fixup for the measure-zero all-zero-Vin rows (reference uses
    # marginals[v] as the logit there).  Pure indexing + O(NS*B) host math.
    V32 = np.asarray(V, np.float32)
    M, vmat, tmat, mch = aux["M"], aux["vmat"], aux["tmat"], aux["mch"]
    base = V32[:, :XDIM].sum(1)                                   # [B]
    zmask = np.zeros((NS, B), bool)
    Mh = M[:, XDIM:]                                              # [V, 896]
    for s in range(NS):
        extra = np.einsum('bc,bc->b', V32[:, XDIM:], Mh[vmat[s]])
        zmask[s] = (base + extra) == 0.0
    if zmask.any():
        marg = np.asarray(marginals, np.float32)
        qm = marg[vmat]                                           # [NS, B]
        sp = np.maximum(qm, 0) + np.log1p(np.exp(-np.abs(qm)))
        ll_m = tmat * qm - sp
        delta = (ll_m - llv) * zmask
        out[0] += delta[0]
        out[1] += (delta[1:] * mch[1:]).sum(0)
    return out.astype(np.float32)


if __name__ == "__main__":
    d = np.load("/root/problem/ref_data.npz")
    I = {k: d[k] for k in d.files if k != "expected"}
    got = kernel(**I)
    exp = d["expected"]
    err = np.abs(got - exp)
    rel = np.linalg.norm(got - exp) / np.linalg.norm(exp)
    print("max abs", err.max(), "l2 rel", rel)


# revision 19
# speedup vs baseline: 1.0014x; 1.0014x over previous
"""Trainium2 Bass kernel for nn_DeltaAI_84061099918079 (gnn_message_passing).

Math reformulation of the reference:
  For each batch row b with i = ilist[b], the 9 qnet evaluations (1 self +
  8 children) all use Vin = V[b] * M[v] where M[v, c] = (c < 128 or
  c in K_pa[v]) is one of only 1024 distinct masks, and v = i (slot 0) or
  v = K_ch[i, s-1] (slots 1..8).  bern_logprob(q, t) == t*q - softplus(q).
  elu(x) == relu(x) + min(exp(x), 1) - 1.

Optimizations on top of the straightforward mapping:
  - Weight columns are centered host-side (W~ = W - colmean): the LN mean of
    x = W^T v + b is then exactly 0, so no mean stats and no mean subtract.
  - LN gain g is folded into W (stats selector carries 1/(H g^2)); LN beta and
    layer biases are zero in this problem so those adds are skipped (checked).
  - Activations are stored shifted (h' = h+1) so ELU needs no "-1" pass; the
    shift is compensated in the next layer's bias and the head bias table.
  - Matmuls run slot-inner so consecutive matmuls share Ldweights.
  - Elementwise work is spread: DVE uses 2x-mode tensor_tensor ops only,
    PSUM->SBUF copies and the L1 ELU combine run on GPSIMD/Pool.
"""

import os
import sys
import numpy as np

sys.path.insert(0, "/opt/trn_rl_repo")

import ml_dtypes

bf16 = ml_dtypes.bfloat16

B, VDIM, XDIM, HDIM = 4096, 1024, 128, 512
MAXPA, MAXCH = 8, 8
LN_EPS = 1e-5
NCORES = 8
BSH = B // NCORES          # 512 batch rows per core
NS = 1 + MAXCH             # 9 slots
N = BSH                    # tile columns
KC_V = VDIM // 128         # 8
KC_H = HDIM // 128         # 4

_PROGRAM = {}              # cached per structure-flags


def _build_program(bias_zero, be_zero):
    import concourse.bass as bass
    import concourse.mybir as mybir
    import concourse.tile as tile
    from concourse import bacc
    from contextlib import ExitStack

    FP32 = mybir.dt.float32
    BF16 = mybir.dt.bfloat16
    I16 = mybir.dt.int16
    AF = mybir.ActivationFunctionType
    ALU = mybir.AluOpType
    ts = bass.ts

    nc = bacc.Bacc("TRN2")

    # ---- DRAM tensors ----
    vt_d = nc.dram_tensor("vt", [128, KC_V, N], BF16, kind="ExternalInput")
    mrows_d = nc.dram_tensor("mrows", [VDIM, VDIM], BF16, kind="ExternalInput")
    hwrows_d = nc.dram_tensor("hwrows", [VDIM, HDIM], BF16, kind="ExternalInput")
    w1_d = nc.dram_tensor("w1", [128, KC_V, HDIM], BF16, kind="ExternalInput")
    w2_d = nc.dram_tensor("w2", [128, KC_H, HDIM], BF16, kind="ExternalInput")
    w3_d = nc.dram_tensor("w3", [128, KC_H, HDIM], BF16, kind="ExternalInput")
    if not bias_zero:
        bprm_d = nc.dram_tensor("bprm", [128, 3, KC_H], FP32, kind="ExternalInput")
    if not be_zero:
        beprm_d = nc.dram_tensor("beprm", [128, 3, KC_H], FP32, kind="ExternalInput")
    # stats selector: tile (li,k,j) has col j = 1/(H g^2) weights, rest zero
    selq_d = nc.dram_tensor("selq", [128, 3 * KC_H * 3, 16], BF16, kind="ExternalInput")
    idx_d = nc.dram_tensor("idx", [128, NS, N // 16], I16, kind="ExternalInput")
    tmat_d = nc.dram_tensor("tmat", [NS, N], FP32, kind="ExternalInput")
    mch_d = nc.dram_tensor("mch", [NS, N], FP32, kind="ExternalInput")
    hbg_d = nc.dram_tensor("hbg", [NS, N], FP32, kind="ExternalInput")
    sel_d = nc.dram_tensor("sel", [128, 2 * NS, 64], BF16, kind="ExternalInput")
    fin_d = nc.dram_tensor("fin", [16, 2], FP32, kind="ExternalInput")
    out_d = nc.dram_tensor("out", [2, N], FP32, kind="ExternalOutput")
    llout_d = nc.dram_tensor("llout", [NS, N], FP32, kind="ExternalOutput")

    with tile.TileContext(nc) as tc, ExitStack() as ctx:
        const = ctx.enter_context(tc.tile_pool(name="const", bufs=1))
        hA = ctx.enter_context(tc.tile_pool(name="hA", bufs=1))
        hB = ctx.enter_context(tc.tile_pool(name="hB", bufs=1))
        mgp = ctx.enter_context(tc.tile_pool(name="mgp", bufs=2))
        sqp = ctx.enter_context(tc.tile_pool(name="sqp", bufs=2))
        tmp = ctx.enter_context(tc.tile_pool(name="tmp", bufs=6))
        hwp = ctx.enter_context(tc.tile_pool(name="hwp", bufs=2))
        mbp = ctx.enter_context(tc.tile_pool(name="mbp", bufs=3))
        smp = ctx.enter_context(tc.tile_pool(name="smp", bufs=1))
        xps = ctx.enter_context(
            tc.tile_pool(name="xps", bufs=6, space=bass.MemorySpace.PSUM))
        stp = ctx.enter_context(
            tc.tile_pool(name="stp", bufs=1, space=bass.MemorySpace.PSUM))
        qps = ctx.enter_context(
            tc.tile_pool(name="qps", bufs=1, space=bass.MemorySpace.PSUM))

        # ---- load constants ----
        _eng = [nc.sync, nc.gpsimd, nc.scalar]
        _engi = [0]

        def load(shape, dt, src, tag):
            t = const.tile(shape, dt, tag=tag, name=tag)
            _eng[_engi[0] % len(_eng)].dma_start(t[:], src[:])
            _engi[0] += 1
            return t

        idxa = load([128, NS, N // 16], I16, idx_d, "idxa")
        vt = load([128, KC_V, N], BF16, vt_d, "vt")
        w1 = load([128, KC_V, HDIM], BF16, w1_d, "w1")
        w2 = load([128, KC_H, HDIM], BF16, w2_d, "w2")
        w3 = load([128, KC_H, HDIM], BF16, w3_d, "w3")
        selq = load([128, 3 * KC_H * 3, 16], BF16, selq_d, "selq")
        tmat = load([NS, N], FP32, tmat_d, "tmat")
        mch = load([NS, N], FP32, mch_d, "mch")
        hbg = load([NS, N], FP32, hbg_d, "hbg")
        sel = load([128, 2 * NS, 64], BF16, sel_d, "sel")
        fin = load([16, 2], FP32, fin_d, "fin")
        if not bias_zero:
            bprm = load([128, 3, KC_H], FP32, bprm_d, "bprm")
        if not be_zero:
            beprm = load([128, 3, KC_H], FP32, beprm_d, "beprm")
        idxt = [idxa[:, s, :] for s in range(NS)]
        epst = const.tile([NS, 1], FP32, tag="epst", name="epst")
        nc.vector.memset(epst[:], LN_EPS)
        onet = const.tile([NS, 1], FP32, tag="onet", name="onet")
        nc.vector.memset(onet[:], 1.0)
        onep = const.tile([128, 1], FP32, tag="onep", name="onep")
        nc.vector.memset(onep[:], 1.0)
        zt = const.tile([128, KC_H, N], BF16, tag="zt", name="zt")
        nc.vector.memset(zt[:], 0.0)

        ws = [w1, w2, w3]
        kcs = [KC_V, KC_H, KC_H]

        hAt = [hA.tile([128, KC_H, N], BF16, tag=f"hA{s}", name=f"hA{s}") for s in range(NS)]
        hBt = [hB.tile([128, KC_H, N], BF16, tag=f"hB{s}", name=f"hB{s}") for s in range(NS)]

        # ---- Phase 0: per-slot masked inputs vin = V^T * M[v]^T ----
        vin_t = []
        for s in range(NS):
            mg = mgp.tile([128, KC_V, N], BF16, tag="mg")
            nc.gpsimd.dma_gather(
                mg[:], mrows_d[:], idxt[s][:], N, N, VDIM, transpose=True)
            nc.vector.tensor_mul(mg[:], vt[:], mg[:])
            vin_t.append(mg)

        # ---- layers ----
        def run_layer(li, inputs, houts, hres):
            """x~ = W~g^T @ inputs (zero-mean by construction); per slot:
            h' = relu(y) + min(exp(y),1) [+ hres] with y = x~ * rstd."""
            w, kc = ws[li], kcs[li]
            for gi, grp in enumerate(([0, 1, 2], [3, 4, 5], [6, 7, 8])):
              # main matmuls
              for j, s in enumerate(grp):
                for m in range(KC_H):
                    xp = xps.tile([128, N], mybir.dt.float32, tag="xp",
                                  name=f"xp{li}{gi}{m}{j}")
                    for k in range(kc):
                        nc.tensor.matmul(
                            xp[:], w[:, k, ts(m, 128)], inputs[s][:, k, :],
                            start=(k == 0), stop=(k == kc - 1))
                    if bias_zero:
                        nc.scalar.activation(
                            houts[s][:, m, :], xp[:], AF.Identity)
                    else:
                        nc.scalar.activation(
                            houts[s][:, m, :], xp[:], AF.Identity,
                            bias=bprm[:, li, m:m + 1])

              # variance stats: E[x~^2] via selector matmuls (row j of stat)
              stat = stp.tile([16, N], mybir.dt.float32, tag="stat",
                              name=f"stat{li}{gi}")
              sqs = []
              for j, s in enumerate(grp):
                sq = sqp.tile([128, KC_H, N], BF16, tag="sq",
                              name=f"sq{li}{s}")
                nc.vector.tensor_mul(sq[:], houts[s][:], houts[s][:])
                sqs.append(sq)
              for j, s in enumerate(grp):
                for k in range(KC_H):
                    nc.tensor.matmul(
                        stat[:], selq[:, (li * KC_H + k) * 3 + j, :],
                        sqs[j][:, k, :],
                        start=(j == 0 and k == 0),
                        stop=(j == 2 and k == KC_H - 1),
                        skip_group_check=True)

              # rstd = exp(-0.5*ln(var+eps)) on [3, N] rows
              lnv = smp.tile([NS, N], mybir.dt.float32, tag="lnv",
                             name=f"lnv{li}{gi}")[0:3, :]
              nc.scalar.activation(lnv[:], stat[0:3, :], AF.Ln, bias=epst[0:3])
              mrp = smp.tile([NS, 1, N], BF16, tag="mrp",
                             name=f"mrp{li}{gi}")
              nc.scalar.activation(mrp[0:3, 0, :], lnv[:], AF.Exp, scale=-0.5)

              for j, s in enumerate(grp):
                xs = houts[s]
                mrps = mbp.tile([1, 1, N], BF16, tag="mrps",
                                name=f"mrps{li}{s}")
                nc.sync.dma_start(mrps[:], mrp[j:j + 1, :, :])
                mb = mbp.tile([128, 1, N], BF16, tag="mb")
                nc.gpsimd.partition_broadcast(mb[:], mrps[:])
                r_b = mb[:, 0:1, :].broadcast_to([128, KC_H, N])
                yy = tmp.tile([128, KC_H, N], BF16, tag="tmp")
                nc.vector.tensor_mul(yy[:], xs[:], r_b)
                if not be_zero:
                    for m in range(KC_H):
                        nc.gpsimd.tensor_scalar(
                            yy[:, m, :], yy[:, m, :],
                            beprm[:, li, m:m + 1], None, op0=ALU.add)
                ee = tmp.tile([128, KC_H, N], BF16, tag="tmp")
                nc.scalar.activation(ee[:], yy[:], AF.Exp)
                rl = tmp.tile([128, KC_H, N], BF16, tag="tmp")
                nc.vector.tensor_max(rl[:], yy[:], zt[:])
                if hres is None:
                    # h1' = min(ee,1) + relu(y)
                    nc.vector.scalar_tensor_tensor(
                        xs[:], ee[:], 1.0, rl[:], op0=ALU.min, op1=ALU.add)
                else:
                    # h' = hres' + relu(y) - relu(1-ee)
                    tt = tmp.tile([128, KC_H, N], BF16, tag="tmp")
                    nc.scalar.activation(tt[:], ee[:], AF.Relu,
                                         bias=onep[:], scale=-1.0)
                    vv = tmp.tile([128, KC_H, N], BF16, tag="tmp")
                    nc.vector.tensor_sub(vv[:], rl[:], tt[:])
                    nc.vector.tensor_add(xs[:], vv[:], hres[s][:])

        run_layer(0, vin_t, hAt, None)          # h1' in hAt
        run_layer(1, hAt, hBt, hAt)             # h2' in hBt
        run_layer(2, hBt, hAt, hBt)             # h3' in hAt

        # ---- head: q[s, n] = sum_h h3'[h,n]*headW[v][h]  (shift fixed in hbg)
        qp = qps.tile([32, N], mybir.dt.float32, tag="q")
        for s in range(NS):
            hw = hwp.tile([128, KC_H, N], BF16, tag="hw")
            nc.gpsimd.dma_gather(
                hw[:], hwrows_d[:], idxt[s][:], N, N, HDIM, transpose=True)
            nc.vector.tensor_mul(hw[:], hAt[s][:], hw[:])
            for k in range(KC_H):
                nc.tensor.matmul(
                    qp[:], sel[:, 6 + s, 0:32], hw[:, k, :],
                    start=(s == 0 and k == 0),
                    stop=(s == NS - 1 and k == KC_H - 1),
                    skip_group_check=True)

        # ---- bern ll + child sum ----
        q2 = smp.tile([NS, N], mybir.dt.float32, tag="q2")
        nc.vector.scalar_tensor_tensor(
            q2[:], qp[0:NS, :], 1.0, hbg[:], op0=ALU.mult, op1=ALU.add)
        # softplus(q) = relu(q) + ln(1 + exp(-|q|))
        aq = smp.tile([NS, N], mybir.dt.float32, tag="aq")
        nc.scalar.activation(aq[:], q2[:], AF.Abs)
        eq = smp.tile([NS, N], mybir.dt.float32, tag="eq")
        nc.scalar.activation(eq[:], aq[:], AF.Exp, scale=-1.0)
        lg = smp.tile([NS, N], mybir.dt.float32, tag="lg")
        nc.scalar.activation(lg[:], eq[:], AF.Ln, bias=onet[:])
        rq = smp.tile([NS, N], mybir.dt.float32, tag="rq")
        nc.vector.tensor_scalar_max(rq[:], q2[:], 0.0)
        sp = smp.tile([NS, N], mybir.dt.float32, tag="sp")
        nc.vector.tensor_add(sp[:], rq[:], lg[:])
        tq = smp.tile([NS, N], mybir.dt.float32, tag="tq")
        nc.vector.tensor_mul(tq[:], tmat[:], q2[:])
        llv = smp.tile([NS, N], mybir.dt.float32, tag="llv")
        nc.vector.scalar_tensor_tensor(
            llv[:], sp[:], -1.0, tq[:], op0=ALU.mult, op1=ALU.add)
        llm = const.tile([16, N], mybir.dt.float32, tag="llm")
        nc.vector.memset(llm[:], 0.0)
        nc.vector.tensor_mul(llm[0:NS, :], llv[:], mch[:])
        fo = qps.tile([32, N], mybir.dt.float32, tag="q", name="fo")[0:2, :]
        nc.tensor.matmul(fo[:], fin[:], llm[:], start=True, stop=True)
        ob = smp.tile([2, N], mybir.dt.float32, tag="ob")
        nc.vector.tensor_copy(ob[:], fo[:])
        nc.sync.dma_start(out_d[:], ob[:])
        nc.sync.dma_start(llout_d[:], llv[:])

    nc.compile()
    return nc


def _get_program(bias_zero=True, be_zero=True):
    key = (bias_zero, be_zero)
    if key not in _PROGRAM:
        _PROGRAM[key] = _build_program(bias_zero, be_zero)
    return _PROGRAM[key]


def _host_prep(V, K_pa, K_ch, ilist, W1, W2, W3, b1, g1, be1, b2, g2, be2,
               b3, g3, be3, headW, headb):
    """Index-derived tables + sharded/replicated device buffers."""
    V = np.asarray(V, np.float32)
    K_pa = np.asarray(K_pa).astype(np.int64)
    K_ch = np.asarray(K_ch).astype(np.int64)
    ilist = np.asarray(ilist).astype(np.int64)

    # mask matrix M[v, c] (bf16 exact 0/1)
    M = np.zeros((VDIM, VDIM), np.float32)
    M[:, :XDIM] = 1.0
    vr = np.repeat(np.arange(VDIM), MAXPA)
    pa = K_pa.ravel()
    ok = pa >= 0
    M[vr[ok], pa[ok]] = 1.0

    # node index per (slot, batch-row)
    vmat = np.zeros((NS, B), np.int64)
    vmat[0] = ilist
    ch = K_ch[ilist]                     # [B, 8]
    ch_ok = ch >= 0
    vmat[1:] = np.where(ch_ok, ch, 0).T  # [8, B]

    tmat = V[np.arange(B)[None, :], vmat].astype(np.float32)      # [NS, B]
    mch = np.ones((NS, B), np.float32)
    mch[1:] = ch_ok.T.astype(np.float32)
    # head bias with the h'=h+1 shift correction folded in
    hw_rs = np.asarray(headW, np.float32).sum(1)                  # [VDIM]
    hbg = (np.asarray(headb, np.float32) - hw_rs)[vmat]           # [NS, B]

    # center weight columns (LN mean -> 0), fold gain g into W, and fold the
    # h'=h+1 input shift of layers 2/3 into their (centered) biases
    gs = [np.asarray(g, np.float32) for g in (g1, g2, g3)]
    bs = [np.asarray(b, np.float32) for b in (b1, b2, b3)]
    bes = [np.asarray(be, np.float32) for be in (be1, be2, be3)]
    Wg, bias_l = [], []
    for li, W in enumerate((W1, W2, W3)):
        W = np.asarray(W, np.float32)
        Wc = (W - W.mean(1, keepdims=True)) * gs[li][None, :]
        bc = (bs[li] - bs[li].mean()) * gs[li]
        if li > 0:
            bc = bc - Wc.sum(0)          # input is h_prev + 1
        Wg.append(Wc)
        bias_l.append(bc)

    bias_zero = all(np.abs(b).max() < 1e-30 for b in bias_l)
    be_zero = all(np.abs(be).max() < 1e-30 for be in bes)

    def chunk_feat(w, kc):
        return np.ascontiguousarray(
            np.asarray(w, np.float32).reshape(kc, 128, -1).transpose(1, 0, 2)
        ).astype(bf16)

    w1c = chunk_feat(Wg[0], KC_V)
    w2c = chunk_feat(Wg[1], KC_H)
    w3c = chunk_feat(Wg[2], KC_H)

    def chunk_param(*ps):
        return np.ascontiguousarray(np.stack(
            [np.asarray(p, np.float32).reshape(KC_H, 128).T for p in ps],
            axis=1))

    bprm = chunk_param(*bias_l)
    beprm = chunk_param(*bes)

    # stats selector: tile (li,k,j) has only col j nonzero = 1/(H g^2)
    selq = np.zeros((128, 3 * KC_H * 3, 16), np.float32)
    for li in range(3):
        wv = 1.0 / (HDIM * gs[li] ** 2)              # [HDIM]
        for k in range(KC_H):
            for j in range(3):
                selq[:, (li * KC_H + k) * 3 + j, j] = wv[k * 128:(k + 1) * 128]

    sel = np.zeros((128, 2 * NS, 64), np.float32)
    for s in range(NS):
        sel[:, 6 + s, s] = 1.0
    fin = np.zeros((16, 2), np.float32)
    fin[0, 0] = 1.0
    fin[1:NS, 1] = 1.0

    Mb = M.astype(bf16)
    HWb = np.asarray(headW, np.float32).astype(bf16)

    in_maps = []
    for c in range(NCORES):
        rows = slice(c * BSH, (c + 1) * BSH)
        vt = np.ascontiguousarray(
            V[rows].T.reshape(KC_V, 128, BSH).transpose(1, 0, 2)).astype(bf16)
        vm = vmat[:, rows]                                        # [NS, 512]
        idx = np.zeros((128, NS, N // 16), np.int16)
        for s in range(NS):
            wrapped = vm[s].reshape(N // 16, 16).T.astype(np.int16)
            idx[:, s, :] = np.tile(wrapped, (8, 1))
        im = dict(
            vt=vt, mrows=Mb, hwrows=HWb, w1=w1c, w2=w2c, w3=w3c,
            selq=selq.astype(bf16), idx=idx,
            tmat=np.ascontiguousarray(tmat[:, rows]),
            mch=np.ascontiguousarray(mch[:, rows]),
            hbg=np.ascontiguousarray(hbg[:, rows]),
            sel=sel.astype(bf16), fin=fin,
        )
        if not bias_zero:
            im["bprm"] = bprm
        if not be_zero:
            im["beprm"] = beprm
        in_maps.append(im)

    aux = dict(M=M, vmat=vmat, tmat=tmat, mch=mch,
               bias_zero=bias_zero, be_zero=be_zero)
    return in_maps, aux


def kernel(V, K_pa, K_ch, ilist, W1, b1, g1, be1, W2, b2, g2, be2,
           W3, b3, g3, be3, headW, headb, marginals):
    from concourse.bass_utils import run_bass_kernel_spmd

    in_maps, aux = _host_prep(V, K_pa, K_ch, ilist, W1, W2, W3, b1, g1, be1,
                              b2, g2, be2, b3, g3, be3, headW, headb)
    nc = _get_program(aux["bias_zero"], aux["be_zero"])
    res = run_bass_kernel_spmd(nc, in_maps, core_ids=list(range(NCORES)))
    out = np.concatenate([r["out"] for r in res.results], axis=1)  # [2, B]
    llv = np.concatenate([r["llout"] for r in res.results], axis=1)  # [NS, B]

    # Exact fixup for the measure-zero all-zero-Vin rows (reference uses
    # marginals[v] as the logit there).  Pure indexing + O(NS*B) host math.
    V32 = np.asarray(V, np.float32)
    M, vmat, tmat, mch = aux["M"], aux["vmat"], aux["tmat"], aux["mch"]
    base = V32[:, :XDIM].sum(1)                                   # [B]
    zmask = np.zeros((NS, B), bool)
    Mh = M[:, XDIM:]                                              # [V, 896]
    for s in range(NS):
        extra = np.einsum('bc,bc->b', V32[:, XDIM:], Mh[vmat[s]])
        zmask[s] = (base + extra) == 0.0
    if zmask.any():
        marg = np.asarray(marginals, np.float32)
        qm = marg[vmat]                                           # [NS, B]
        sp = np.maximum(qm, 0) + np.log1p(np.exp(-np.abs(qm)))
        ll_m = tmat * qm - sp
        delta = (ll_m - llv) * zmask
        out[0] += delta[0]
        out[1] += (delta[1:] * mch[1:]).sum(0)
    return out.astype(np.float32)


if __name__ == "__main__":
    d = np.load("/root/problem/ref_data.npz")
    I = {k: d[k] for k in d.files if k != "expected"}
    got = kernel(**I)
    exp = d["expected"]
    err = np.abs(got - exp)
    rel = np.linalg.norm(got - exp) / np.linalg.norm(exp)
    print("max abs", err.max(), "l2 rel", rel)


# revision 21
# speedup vs baseline: 1.0048x; 1.0034x over previous
"""Trainium2 Bass kernel for nn_DeltaAI_84061099918079 (gnn_message_passing).

Math reformulation of the reference:
  For each batch row b with i = ilist[b], the 9 qnet evaluations (1 self +
  8 children) all use Vin = V[b] * M[v] where M[v, c] = (c < 128 or
  c in K_pa[v]) is one of only 1024 distinct masks, and v = i (slot 0) or
  v = K_ch[i, s-1] (slots 1..8).  bern_logprob(q, t) == t*q - softplus(q).
  elu(x) == relu(x) + min(exp(x), 1) - 1.

Optimizations on top of the straightforward mapping:
  - Weight columns are centered host-side (W~ = W - colmean): the LN mean of
    x = W^T v + b is then exactly 0, so no mean stats and no mean subtract.
  - LN gain g is folded into W (stats selector carries 1/(H g^2)); LN beta and
    layer biases are zero in this problem so those adds are skipped (checked).
  - Activations are stored shifted (h' = h+1) so ELU needs no "-1" pass; the
    shift is compensated in the next layer's bias and the head bias table.
  - Matmuls run slot-inner so consecutive matmuls share Ldweights.
  - Elementwise work is spread: DVE uses 2x-mode tensor_tensor ops only,
    PSUM->SBUF copies and the L1 ELU combine run on GPSIMD/Pool.
"""

import os
import sys
import numpy as np

sys.path.insert(0, "/opt/trn_rl_repo")

import ml_dtypes

bf16 = ml_dtypes.bfloat16

B, VDIM, XDIM, HDIM = 4096, 1024, 128, 512
MAXPA, MAXCH = 8, 8
LN_EPS = 1e-5
NCORES = 8
BSH = B // NCORES          # 512 batch rows per core
NS = 1 + MAXCH             # 9 slots
N = BSH                    # tile columns
KC_V = VDIM // 128         # 8
KC_H = HDIM // 128         # 4

_PROGRAM = {}              # cached per structure-flags


def _build_program(bias_zero, be_zero):
    import concourse.bass as bass
    import concourse.mybir as mybir
    import concourse.tile as tile
    from concourse import bacc
    from contextlib import ExitStack

    FP32 = mybir.dt.float32
    BF16 = mybir.dt.bfloat16
    I16 = mybir.dt.int16
    AF = mybir.ActivationFunctionType
    ALU = mybir.AluOpType
    ts = bass.ts

    nc = bacc.Bacc("TRN2")

    # ---- DRAM tensors ----
    vt_d = nc.dram_tensor("vt", [128, KC_V, N], BF16, kind="ExternalInput")
    mrows_d = nc.dram_tensor("mrows", [VDIM, VDIM], BF16, kind="ExternalInput")
    hwrows_d = nc.dram_tensor("hwrows", [VDIM, HDIM], BF16, kind="ExternalInput")
    w1_d = nc.dram_tensor("w1", [128, KC_V, HDIM], BF16, kind="ExternalInput")
    w2_d = nc.dram_tensor("w2", [128, KC_H, HDIM], BF16, kind="ExternalInput")
    w3_d = nc.dram_tensor("w3", [128, KC_H, HDIM], BF16, kind="ExternalInput")
    if not bias_zero:
        bprm_d = nc.dram_tensor("bprm", [128, 3, KC_H], FP32, kind="ExternalInput")
    if not be_zero:
        beprm_d = nc.dram_tensor("beprm", [128, 3, KC_H], FP32, kind="ExternalInput")
    # stats selector: tile (li,k,j) has col j = 1/(H g^2) weights, rest zero
    selq_d = nc.dram_tensor("selq", [128, 3 * KC_H * 3, 16], BF16, kind="ExternalInput")
    idx_d = nc.dram_tensor("idx", [128, NS, N // 16], I16, kind="ExternalInput")
    tmat_d = nc.dram_tensor("tmat", [NS, N], FP32, kind="ExternalInput")
    mch_d = nc.dram_tensor("mch", [NS, N], FP32, kind="ExternalInput")
    hbg_d = nc.dram_tensor("hbg", [NS, N], FP32, kind="ExternalInput")
    sel_d = nc.dram_tensor("sel", [128, 2 * NS, 64], BF16, kind="ExternalInput")
    fin_d = nc.dram_tensor("fin", [16, 2], FP32, kind="ExternalInput")
    out_d = nc.dram_tensor("out", [2, N], FP32, kind="ExternalOutput")
    llout_d = nc.dram_tensor("llout", [NS, N], FP32, kind="ExternalOutput")

    with tile.TileContext(nc) as tc, ExitStack() as ctx:
        const = ctx.enter_context(tc.tile_pool(name="const", bufs=1))
        hA = ctx.enter_context(tc.tile_pool(name="hA", bufs=1))
        hB = ctx.enter_context(tc.tile_pool(name="hB", bufs=1))
        mgp = ctx.enter_context(tc.tile_pool(name="mgp", bufs=2))
        sqp = ctx.enter_context(tc.tile_pool(name="sqp", bufs=2))
        tmp = ctx.enter_context(tc.tile_pool(name="tmp", bufs=6))
        hwp = ctx.enter_context(tc.tile_pool(name="hwp", bufs=2))
        mbp = ctx.enter_context(tc.tile_pool(name="mbp", bufs=3))
        smp = ctx.enter_context(tc.tile_pool(name="smp", bufs=1))
        xps = ctx.enter_context(
            tc.tile_pool(name="xps", bufs=6, space=bass.MemorySpace.PSUM))
        stp = ctx.enter_context(
            tc.tile_pool(name="stp", bufs=1, space=bass.MemorySpace.PSUM))
        qps = ctx.enter_context(
            tc.tile_pool(name="qps", bufs=1, space=bass.MemorySpace.PSUM))

        # ---- load constants ----
        _eng = [nc.sync, nc.gpsimd, nc.scalar]
        _engi = [0]

        def load(shape, dt, src, tag):
            t = const.tile(shape, dt, tag=tag, name=tag)
            _eng[_engi[0] % len(_eng)].dma_start(t[:], src[:])
            _engi[0] += 1
            return t

        idxa = load([128, NS, N // 16], I16, idx_d, "idxa")
        vt = load([128, KC_V, N], BF16, vt_d, "vt")
        w1 = load([128, KC_V, HDIM], BF16, w1_d, "w1")
        w2 = load([128, KC_H, HDIM], BF16, w2_d, "w2")
        w3 = load([128, KC_H, HDIM], BF16, w3_d, "w3")
        selq = load([128, 3 * KC_H * 3, 16], BF16, selq_d, "selq")
        tmat = load([NS, N], FP32, tmat_d, "tmat")
        mch = load([NS, N], FP32, mch_d, "mch")
        hbg = load([NS, N], FP32, hbg_d, "hbg")
        sel = load([128, 2 * NS, 64], BF16, sel_d, "sel")
        fin = load([16, 2], FP32, fin_d, "fin")
        if not bias_zero:
            bprm = load([128, 3, KC_H], FP32, bprm_d, "bprm")
        if not be_zero:
            beprm = load([128, 3, KC_H], FP32, beprm_d, "beprm")
        idxt = [idxa[:, s, :] for s in range(NS)]
        epst = const.tile([NS, 1], FP32, tag="epst", name="epst")
        nc.vector.memset(epst[:], LN_EPS)
        onet = const.tile([NS, 1], FP32, tag="onet", name="onet")
        nc.vector.memset(onet[:], 1.0)
        onep = const.tile([128, 1], FP32, tag="onep", name="onep")
        nc.vector.memset(onep[:], 1.0)
        zt = const.tile([128, KC_H, N], BF16, tag="zt", name="zt")
        nc.vector.memset(zt[:], 0.0)

        ws = [w1, w2, w3]
        kcs = [KC_V, KC_H, KC_H]

        hAt = [hA.tile([128, KC_H, N], BF16, tag=f"hA{s}", name=f"hA{s}") for s in range(NS)]
        hBt = [hB.tile([128, KC_H, N], BF16, tag=f"hB{s}", name=f"hB{s}") for s in range(NS)]

        # ---- Phase 0: per-slot masked inputs vin = V^T * M[v]^T ----
        vin_t = []
        for s in range(NS):
            mg = mgp.tile([128, KC_V, N], BF16, tag="mg")
            nc.gpsimd.dma_gather(
                mg[:], mrows_d[:], idxt[s][:], N, N, VDIM, transpose=True)
            nc.vector.tensor_mul(mg[:], vt[:], mg[:])
            vin_t.append(mg)

        # ---- layers ----
        def run_layer(li, inputs, houts, hres):
            """x~ = W~g^T @ inputs (zero-mean by construction); per slot:
            h' = relu(y) + min(exp(y),1) [+ hres] with y = x~ * rstd."""
            w, kc = ws[li], kcs[li]
            for gi, grp in enumerate(([0, 1, 2], [3, 4, 5], [6, 7, 8])):
              # main matmuls
              for j, s in enumerate(grp):
                for m in range(KC_H):
                    xp = xps.tile([128, N], mybir.dt.float32, tag="xp",
                                  name=f"xp{li}{gi}{m}{j}")
                    for k in range(kc):
                        # L1 chunk 0 is never masked (c < 128): read vt
                        # directly so matmuls start before the mask gather
                        rhs = (vt[:, 0, :] if li == 0 and k == 0
                               else inputs[s][:, k, :])
                        nc.tensor.matmul(
                            xp[:], w[:, k, ts(m, 128)], rhs,
                            start=(k == 0), stop=(k == kc - 1))
                    if bias_zero:
                        nc.scalar.activation(
                            houts[s][:, m, :], xp[:], AF.Identity)
                    else:
                        nc.scalar.activation(
                            houts[s][:, m, :], xp[:], AF.Identity,
                            bias=bprm[:, li, m:m + 1])

              # variance stats: E[x~^2] via selector matmuls (row j of stat)
              stat = stp.tile([16, N], mybir.dt.float32, tag="stat",
                              name=f"stat{li}{gi}")
              sqs = []
              for j, s in enumerate(grp):
                sq = sqp.tile([128, KC_H, N], BF16, tag="sq",
                              name=f"sq{li}{s}")
                nc.vector.tensor_mul(sq[:], houts[s][:], houts[s][:])
                sqs.append(sq)
              for j, s in enumerate(grp):
                for k in range(KC_H):
                    nc.tensor.matmul(
                        stat[:], selq[:, (li * KC_H + k) * 3 + j, :],
                        sqs[j][:, k, :],
                        start=(j == 0 and k == 0),
                        stop=(j == 2 and k == KC_H - 1),
                        skip_group_check=True)

              # rstd = exp(-0.5*ln(var+eps)) on [3, N] rows
              lnv = smp.tile([NS, N], mybir.dt.float32, tag="lnv",
                             name=f"lnv{li}{gi}")[0:3, :]
              nc.scalar.activation(lnv[:], stat[0:3, :], AF.Ln, bias=epst[0:3])
              mrp = smp.tile([NS, 1, N], BF16, tag="mrp",
                             name=f"mrp{li}{gi}")
              nc.scalar.activation(mrp[0:3, 0, :], lnv[:], AF.Exp, scale=-0.5)

              for j, s in enumerate(grp):
                xs = houts[s]
                mrps = mbp.tile([1, 1, N], BF16, tag="mrps",
                                name=f"mrps{li}{s}")
                nc.sync.dma_start(mrps[:], mrp[j:j + 1, :, :])
                mb = mbp.tile([128, 1, N], BF16, tag="mb")
                nc.gpsimd.partition_broadcast(mb[:], mrps[:])
                r_b = mb[:, 0:1, :].broadcast_to([128, KC_H, N])
                yy = tmp.tile([128, KC_H, N], BF16, tag="tmp")
                nc.vector.tensor_mul(yy[:], xs[:], r_b)
                if not be_zero:
                    for m in range(KC_H):
                        nc.gpsimd.tensor_scalar(
                            yy[:, m, :], yy[:, m, :],
                            beprm[:, li, m:m + 1], None, op0=ALU.add)
                ee = tmp.tile([128, KC_H, N], BF16, tag="tmp")
                nc.scalar.activation(ee[:], yy[:], AF.Exp)
                rl = tmp.tile([128, KC_H, N], BF16, tag="tmp")
                nc.vector.tensor_max(rl[:], yy[:], zt[:])
                if hres is None:
                    # h1' = min(ee,1) + relu(y)
                    nc.vector.scalar_tensor_tensor(
                        xs[:], ee[:], 1.0, rl[:], op0=ALU.min, op1=ALU.add)
                else:
                    # h' = hres' + relu(y) - relu(1-ee)
                    tt = tmp.tile([128, KC_H, N], BF16, tag="tmp")
                    nc.scalar.activation(tt[:], ee[:], AF.Relu,
                                         bias=onep[:], scale=-1.0)
                    vv = tmp.tile([128, KC_H, N], BF16, tag="tmp")
                    nc.vector.tensor_sub(vv[:], rl[:], tt[:])
                    nc.vector.tensor_add(xs[:], vv[:], hres[s][:])

        run_layer(0, vin_t, hAt, None)          # h1' in hAt

        # hoist head gathers: no compute deps, overlap with layers 2/3
        hw_t = []
        for s in range(NS):
            hw = mgp.tile([128, KC_H, N], BF16, tag="mg", name=f"hw{s}")
            nc.gpsimd.dma_gather(
                hw[:], hwrows_d[:], idxt[s][:], N, N, HDIM, transpose=True)
            hw_t.append(hw)

        run_layer(1, hAt, hBt, hAt)             # h2' in hBt
        run_layer(2, hBt, hAt, hBt)             # h3' in hAt

        # ---- head: q[s, n] = sum_h h3'[h,n]*headW[v][h]  (shift fixed in hbg)
        qp = qps.tile([32, N], mybir.dt.float32, tag="q")
        for s in range(NS):
            hw = hw_t[s]
            nc.vector.tensor_mul(hw[:], hAt[s][:], hw[:])
            for k in range(KC_H):
                nc.tensor.matmul(
                    qp[:], sel[:, 6 + s, 0:32], hw[:, k, :],
                    start=(s == 0 and k == 0),
                    stop=(s == NS - 1 and k == KC_H - 1),
                    skip_group_check=True)

        # ---- bern ll + child sum ----
        q2 = smp.tile([NS, N], mybir.dt.float32, tag="q2")
        nc.vector.scalar_tensor_tensor(
            q2[:], qp[0:NS, :], 1.0, hbg[:], op0=ALU.mult, op1=ALU.add)
        # softplus(q) = relu(q) + ln(1 + exp(-|q|))
        aq = smp.tile([NS, N], mybir.dt.float32, tag="aq")
        nc.scalar.activation(aq[:], q2[:], AF.Abs)
        eq = smp.tile([NS, N], mybir.dt.float32, tag="eq")
        nc.scalar.activation(eq[:], aq[:], AF.Exp, scale=-1.0)
        lg = smp.tile([NS, N], mybir.dt.float32, tag="lg")
        nc.scalar.activation(lg[:], eq[:], AF.Ln, bias=onet[:])
        rq = smp.tile([NS, N], mybir.dt.float32, tag="rq")
        nc.vector.tensor_scalar_max(rq[:], q2[:], 0.0)
        sp = smp.tile([NS, N], mybir.dt.float32, tag="sp")
        nc.vector.tensor_add(sp[:], rq[:], lg[:])
        tq = smp.tile([NS, N], mybir.dt.float32, tag="tq")
        nc.vector.tensor_mul(tq[:], tmat[:], q2[:])
        llv = smp.tile([NS, N], mybir.dt.float32, tag="llv")
        nc.vector.scalar_tensor_tensor(
            llv[:], sp[:], -1.0, tq[:], op0=ALU.mult, op1=ALU.add)
        llm = const.tile([16, N], mybir.dt.float32, tag="llm")
        nc.vector.memset(llm[:], 0.0)
        nc.vector.tensor_mul(llm[0:NS, :], llv[:], mch[:])
        fo = qps.tile([32, N], mybir.dt.float32, tag="q", name="fo")[0:2, :]
        nc.tensor.matmul(fo[:], fin[:], llm[:], start=True, stop=True)
        ob = smp.tile([2, N], mybir.dt.float32, tag="ob")
        nc.vector.tensor_copy(ob[:], fo[:])
        nc.sync.dma_start(out_d[:], ob[:])
        nc.sync.dma_start(llout_d[:], llv[:])

    nc.compile()
    return nc


def _get_program(bias_zero=True, be_zero=True):
    key = (bias_zero, be_zero)
    if key not in _PROGRAM:
        _PROGRAM[key] = _build_program(bias_zero, be_zero)
    return _PROGRAM[key]


def _host_prep(V, K_pa, K_ch, ilist, W1, W2, W3, b1, g1, be1, b2, g2, be2,
               b3, g3, be3, headW, headb):
    """Index-derived tables + sharded/replicated device buffers."""
    V = np.asarray(V, np.float32)
    K_pa = np.asarray(K_pa).astype(np.int64)
    K_ch = np.asarray(K_ch).astype(np.int64)
    ilist = np.asarray(ilist).astype(np.int64)

    # mask matrix M[v, c] (bf16 exact 0/1)
    M = np.zeros((VDIM, VDIM), np.float32)
    M[:, :XDIM] = 1.0
    vr = np.repeat(np.arange(VDIM), MAXPA)
    pa = K_pa.ravel()
    ok = pa >= 0
    M[vr[ok], pa[ok]] = 1.0

    # node index per (slot, batch-row)
    vmat = np.zeros((NS, B), np.int64)
    vmat[0] = ilist
    ch = K_ch[ilist]                     # [B, 8]
    ch_ok = ch >= 0
    vmat[1:] = np.where(ch_ok, ch, 0).T  # [8, B]

    tmat = V[np.arange(B)[None, :], vmat].astype(np.float32)      # [NS, B]
    mch = np.ones((NS, B), np.float32)
    mch[1:] = ch_ok.T.astype(np.float32)
    # head bias with the h'=h+1 shift correction folded in
    hw_rs = np.asarray(headW, np.float32).sum(1)                  # [VDIM]
    hbg = (np.asarray(headb, np.float32) - hw_rs)[vmat]           # [NS, B]

    # center weight columns (LN mean -> 0), fold gain g into W, and fold the
    # h'=h+1 input shift of layers 2/3 into their (centered) biases
    gs = [np.asarray(g, np.float32) for g in (g1, g2, g3)]
    bs = [np.asarray(b, np.float32) for b in (b1, b2, b3)]
    bes = [np.asarray(be, np.float32) for be in (be1, be2, be3)]
    Wg, bias_l = [], []
    for li, W in enumerate((W1, W2, W3)):
        W = np.asarray(W, np.float32)
        Wc = (W - W.mean(1, keepdims=True)) * gs[li][None, :]
        bc = (bs[li] - bs[li].mean()) * gs[li]
        if li > 0:
            bc = bc - Wc.sum(0)          # input is h_prev + 1
        Wg.append(Wc)
        bias_l.append(bc)

    bias_zero = all(np.abs(b).max() < 1e-30 for b in bias_l)
    be_zero = all(np.abs(be).max() < 1e-30 for be in bes)

    def chunk_feat(w, kc):
        return np.ascontiguousarray(
            np.asarray(w, np.float32).reshape(kc, 128, -1).transpose(1, 0, 2)
        ).astype(bf16)

    w1c = chunk_feat(Wg[0], KC_V)
    w2c = chunk_feat(Wg[1], KC_H)
    w3c = chunk_feat(Wg[2], KC_H)

    def chunk_param(*ps):
        return np.ascontiguousarray(np.stack(
            [np.asarray(p, np.float32).reshape(KC_H, 128).T for p in ps],
            axis=1))

    bprm = chunk_param(*bias_l)
    beprm = chunk_param(*bes)

    # stats selector: tile (li,k,j) has only col j nonzero = 1/(H g^2)
    selq = np.zeros((128, 3 * KC_H * 3, 16), np.float32)
    for li in range(3):
        wv = 1.0 / (HDIM * gs[li] ** 2)              # [HDIM]
        for k in range(KC_H):
            for j in range(3):
                selq[:, (li * KC_H + k) * 3 + j, j] = wv[k * 128:(k + 1) * 128]

    sel = np.zeros((128, 2 * NS, 64), np.float32)
    for s in range(NS):
        sel[:, 6 + s, s] = 1.0
    fin = np.zeros((16, 2), np.float32)
    fin[0, 0] = 1.0
    fin[1:NS, 1] = 1.0

    Mb = M.astype(bf16)
    HWb = np.asarray(headW, np.float32).astype(bf16)

    in_maps = []
    for c in range(NCORES):
        rows = slice(c * BSH, (c + 1) * BSH)
        vt = np.ascontiguousarray(
            V[rows].T.reshape(KC_V, 128, BSH).transpose(1, 0, 2)).astype(bf16)
        vm = vmat[:, rows]                                        # [NS, 512]
        idx = np.zeros((128, NS, N // 16), np.int16)
        for s in range(NS):
            wrapped = vm[s].reshape(N // 16, 16).T.astype(np.int16)
            idx[:, s, :] = np.tile(wrapped, (8, 1))
        im = dict(
            vt=vt, mrows=Mb, hwrows=HWb, w1=w1c, w2=w2c, w3=w3c,
            selq=selq.astype(bf16), idx=idx,
            tmat=np.ascontiguousarray(tmat[:, rows]),
            mch=np.ascontiguousarray(mch[:, rows]),
            hbg=np.ascontiguousarray(hbg[:, rows]),
            sel=sel.astype(bf16), fin=fin,
        )
        if not bias_zero:
            im["bprm"] = bprm
        if not be_zero:
            im["beprm"] = beprm
        in_maps.append(im)

    aux = dict(M=M, vmat=vmat, tmat=tmat, mch=mch,
               bias_zero=bias_zero, be_zero=be_zero)
    return in_maps, aux


def kernel(V, K_pa, K_ch, ilist, W1, b1, g1, be1, W2, b2, g2, be2,
           W3, b3, g3, be3, headW, headb, marginals):
    from concourse.bass_utils import run_bass_kernel_spmd

    in_maps, aux = _host_prep(V, K_pa, K_ch, ilist, W1, W2, W3, b1, g1, be1,
                              b2, g2, be2, b3, g3, be3, headW, headb)
    nc = _get_program(aux["bias_zero"], aux["be_zero"])
    res = run_bass_kernel_spmd(nc, in_maps, core_ids=list(range(NCORES)))
    out = np.concatenate([r["out"] for r in res.results], axis=1)  # [2, B]
    llv = np.concatenate([r["llout"] for r in res.results], axis=1)  # [NS, B]

    # Exact fixup for the measure-zero all-zero-Vin rows (reference uses
    # marginals[v] as the logit there).  Pure indexing + O(NS*B) host math.
    V32 = np.asarray(V, np.float32)
    M, vmat, tmat, mch = aux["M"], aux["vmat"], aux["tmat"], aux["mch"]
    base = V32[:, :XDIM].sum(1)                                   # [B]
    zmask = np.zeros((NS, B), bool)
    Mh = M[:, XDIM:]                                              # [V, 896]
    for s in range(NS):
        extra = np.einsum('bc,bc->b', V32[:, XDIM:], Mh[vmat[s]])
        zmask[s] = (base + extra) == 0.0
    if zmask.any():
        marg = np.asarray(marginals, np.float32)
        qm = marg[vmat]                                           # [NS, B]
        sp = np.maximum(qm, 0) + np.log1p(np.exp(-np.abs(qm)))
        ll_m = tmat * qm - sp
        delta = (ll_m - llv) * zmask
        out[0] += delta[0]
        out[1] += (delta[1:] * mch[1:]).sum(0)
    return out.astype(np.float32)


if __name__ == "__main__":
    d = np.load("/root/problem/ref_data.npz")
    I = {k: d[k] for k in d.files if k != "expected"}
    got = kernel(**I)
    exp = d["expected"]
    err = np.abs(got - exp)
    rel = np.linalg.norm(got - exp) / np.linalg.norm(exp)
    print("max abs", err.max(), "l2 rel", rel)
